# revision 1
# baseline (speedup 1.0000x reference)
"""DCNv2 Trainium2 kernel, v2: "modulate-x then matmul" dataflow.

Shapes (hardcoded): x [4,128,128,64] f32, kernel [3,3,64,64], bias [64],
offset_kernel [3,3,64,27], offset_bias [27]. Output [4,128,128,64] f32.

Sharding: 8 cores = (batch 4) x (H halves 2). Each core computes 64 output
rows from a 68-row halo'd slab (host-side zero-padded, w-major).

Per core:
  - xt_s[w, (row, c)] bf16 for column shifts s in [-2..2]: five DMA loads
    of the host-prepared w-major slab at shifted partition offsets.
  - x_ch[c, (row, col)] f32 (via PE transpose of xt_0) feeds the offset
    conv: om[27, (rho, w)] by 9 accumulating f32r matmuls; om^T via PE;
    sigmoid + tent algebra -> coef[w, (rho, rt, k)] f32 (mask * tenty * tentx).
  - modulated samples, pixels-on-partitions: per (rho, k):
      samp[w, k*64:+64] = sum_rt coef[w, rho, rt, k] * xt_{kx-1+t}[w, row, :]
    9 MACs; taps split between DVE (tensor_scalar + scalar_tensor_tensor)
    and an ACT(scaled-copy) -> Pool(tensor_tensor add) pipeline.
  - per rho: PE transposes samp [w, 576] -> sampT [(k c), w] (5 chunks,
    Pool-copied PSUM->SBUF), then 5 accumulating matmuls with the main
    kernel prepacked [(k c), o], bias pre-loaded into PSUM by a rank-1
    matmul; output lands [w, (rho, o)] and DMAs out per 4-row block.
"""

import numpy as np

B, H, W, C, CO = 4, 128, 128, 64, 64
KK = 9
PAD = 2                 # vertical halo rows above/below the 64 output rows
ROWS = 64 + 2 * PAD     # 68 slab rows
WPAD = 2                # horizontal zero margin in the host slab
WS = W + 2 * WPAD       # 132 slab columns (w-major partitions in DRAM)
OUT_ROWS = 64
CPAD = 1                # x_ch horizontal pad
XCH_ROWS = 66           # x_ch rows: slab rows 1..66
XCH_W = W + 2 * CPAD    # 130
NKC = KK * C            # 576 contraction size of the main matmul

ACT_TAPS = (6, 7, 8)    # taps combined on the ACT->Pool pipeline

_CACHE = {}


def _build(rep=1):
    import concourse.bass as bass  # noqa: F401
    import concourse.mybir as mybir
    from concourse.tile import TileContext
    from concourse.masks import make_identity

    DVE_TAPS = tuple(k for k in range(KK) if k not in ACT_TAPS)
    OP = mybir.AluOpType
    AF = mybir.ActivationFunctionType
    f32 = mybir.dt.float32
    f32r = mybir.dt.float32r
    bf16 = mybir.dt.bfloat16

    nc = bass.Bass("TRN2")
    xs = nc.dram_tensor("xs", [WS, ROWS * C], bf16, kind="ExternalInput")
    woff = nc.dram_tensor("woff", [C, KK * 27], f32r,
                          kind="ExternalInput")
    wmain = nc.dram_tensor("wmain", [NKC, CO], bf16, kind="ExternalInput")
    obias = nc.dram_tensor("obias", [27, 1], f32, kind="ExternalInput")
    mbias = nc.dram_tensor("mbias", [1, CO], bf16, kind="ExternalInput")
    yout = nc.dram_tensor("yout", [OUT_ROWS * W, CO], f32,
                          kind="ExternalOutput")
    yo3 = yout.rearrange("(r w) c -> w r c", w=W)

    with TileContext(nc) as tc:
        with tc.tile_pool(name="persist", bufs=1) as pp:
            ident = pp.tile([128, 128], bf16)
            make_identity(nc, ident[:])
            woff_sb = pp.tile([C, KK * 27], f32r)
            nc.sync.dma_start(out=woff_sb[:], in_=woff[:, :])
            wm_sb = []
            for ci in range(5):
                p = 128 if ci < 4 else 64
                t = pp.tile([p, CO], bf16, name=f"wm{ci}", tag=f"wm{ci}")
                nc.sync.dma_start(out=t[:],
                                  in_=wmain[ci * 128:ci * 128 + p, :])
                wm_sb.append(t)
            obias_sb = pp.tile([27, 1], f32)
            nc.sync.dma_start(out=obias_sb[:], in_=obias[:, :])
            mbias_sb = pp.tile([1, CO], bf16)
            nc.sync.dma_start(out=mbias_sb[:], in_=mbias[:, :])
            ones_sb = pp.tile([1, 128], bf16)
            nc.gpsimd.memset(ones_sb[:], 1.0)

            xt = [pp.tile([128, ROWS * C], bf16, name=f"xt{s}", tag=f"xt{s}")
                  for s in range(5)]
            x_ch = pp.tile([C, XCH_ROWS * XCH_W], f32r)
            xch3 = x_ch[:].rearrange("c (r w) -> c r w", w=XCH_W)
            omt_sb = pp.tile([128, OUT_ROWS * 27], f32)
            coef_sb = pp.tile([128, OUT_ROWS * 81], f32)
            coef3 = coef_sb[:].rearrange("p (r s q) -> p r s q", s=9, q=KK)

            for _rep in range(rep):
                # ---- load xt variants (shifted w-major slab slices);
                # xt[2] first (it gates x_ch), spread across DGE queues ----
                nc.sync.dma_start(out=xt[1][:], in_=xs[1:1 + 128, :])
                nc.gpsimd.dma_start(out=xt[3][:], in_=xs[3:3 + 128, :])
                nc.sync.dma_start(out=xt[2][:], in_=xs[2:2 + 128, :])
                nc.gpsimd.dma_start(out=xt[0][:], in_=xs[0:128, :])
                nc.sync.dma_start(out=xt[4][:], in_=xs[4:4 + 128, :])


                om3 = omt_sb[:].rearrange("p (r q) -> p r q", q=27)
                with tc.tile_pool(name="xchps", bufs=1, space="PSUM") as xcp, \
                     tc.tile_pool(name="omsb", bufs=2) as om_pool, \
                     tc.tile_pool(name="omps", bufs=1, space="PSUM") as omp, \
                     tc.tile_pool(name="otps", bufs=2, space="PSUM") as otp, \
                     tc.tile_pool(name="coefw", bufs=1) as cw, \
                     tc.tile_pool(name="samps", bufs=1) as sp_pool, \
                     tc.tile_pool(name="acttmp", bufs=1) as at_pool, \
                     tc.tile_pool(name="sampt", bufs=2) as st_pool, \
                     tc.tile_pool(name="outsb", bufs=3) as ou_pool, \
                     tc.tile_pool(name="stps", bufs=1, space="PSUM") as stp, \
                     tc.tile_pool(name="outps", bufs=2, space="PSUM") as op_:
                    samp_slots = [sp_pool.tile([128, 640], bf16,
                                               name=f"samp{i}",
                                               tag=f"samp{i}")
                                  for i in range(8)]
                    for sslot in samp_slots:
                        nc.gpsimd.memset(sslot[:, NKC:640], 0.0)
                    NAT = len(ACT_TAPS)
                    tmps = [at_pool.tile([128, NAT * CO], bf16,
                                         name=f"atmp{i}",
                                         tag=f"atmp{i}") for i in range(3)]
                    NFC = 16 * KK  # 144 coef columns per 16-row chunk

                    def emit_xch_rows(xr0, xr1):
                        # two shifted sources cover the 130-wide padded row
                        # (left half from xt[1] includes x col -1; right half
                        # from xt[3] includes x col 128) with no memset.
                        for g0 in range(xr0, xr1, 2):
                            ptAB = xcp.tile([C, 4 * 128], bf16,
                                            name="ptAB", tag="ptAB")
                            ptA = ptAB[:, 0:256]
                            ptB = ptAB[:, 256:512]
                            for j in range(2):
                                xr = g0 + j
                                nc.tensor.transpose(
                                    ptA[:, j * 128:(j + 1) * 128],
                                    xt[1][:, (xr + 1) * C:(xr + 2) * C],
                                    ident[:])
                                nc.tensor.transpose(
                                    ptB[:, j * 128:(j + 1) * 128],
                                    xt[3][:, (xr + 1) * C:(xr + 2) * C],
                                    ident[:])
                            a3 = ptA.rearrange("c (j w) -> c j w", j=2)
                            b3 = ptB.rearrange("c (j w) -> c j w", j=2)
                            nc.scalar.activation(
                                xch3[:, g0:g0 + 2, 0:65], a3[:, :, 0:65],
                                AF.Copy)
                            nc.scalar.activation(
                                xch3[:, g0:g0 + 2, 65:130], b3[:, :, 63:128],
                                AF.Copy)

                    pom_tiles = {}

                    def emit_om_matmul(g4, om_c, base, k):
                        omv = om_c[:].rearrange("q (r w) -> q r w", w=W)
                        r0 = g4 * 4
                        lr = r0 - base
                        if k == 0:
                            pom_tiles[g4] = omp.tile([27, 4 * W], f32,
                                                     name='pom')
                        pom = pom_tiles[g4]
                        ky, kx = k // 3, k % 3
                        rhs3 = xch3[:, r0 + ky:r0 + ky + 4, kx:kx + W]
                        nc.tensor.matmul(
                            pom[:],
                            woff_sb[:, k * 27:k * 27 + 27],
                            rhs3,
                            start=(k == 0), stop=(k == KK - 1))
                        if k == KK - 1:
                            nc.scalar.activation(
                                omv[:, lr:lr + 4, :], pom[:]
                                .rearrange("q (r w) -> q r w", w=W),
                                AF.Identity, bias=obias_sb[:, :], scale=1.0)
                            del pom_tiles[g4]

                    def emit_omt_granule(g4, om_c, base):
                        g = g4 - base // 4
                        pt = otp.tile([128, 4 * 28], bf16)
                        for j in range(4):
                            lr = g * 4 + j
                            nc.tensor.transpose(
                                pt[:, j * 28:j * 28 + 27],
                                om_c[:, lr * W:(lr + 1) * W],
                                ident[0:27, 0:27])
                        pt3 = pt[:].rearrange("p (j q) -> p j q", q=28)
                        rho0 = g4 * 4
                        nc.vector.tensor_copy(
                            omt_sb[:, rho0 * 27:(rho0 + 4) * 27]
                            .rearrange("p (j q) -> p j q", q=27),
                            pt3[:, :, 0:27])

                    def emit_coef_chunk(lo, hi):
                        nf = (hi - lo) * KK
                        rs = slice(lo, hi)
                        dy_ap = om3[:, rs, 0:9]
                        dx_ap = om3[:, rs, 9:18]
                        mk_raw = om3[:, rs, 18:27]
                        mk_sig = cw.tile([128, NFC], f32, name="mks",
                                         tag="mks")[:, 0:nf]
                        nc.scalar.activation(mk_sig, mk_raw, AF.Sigmoid)
                        tmp = cw.tile([128, NFC], f32, name="ctmp",
                                      tag="ctmp")[:, 0:nf]
                        tmpb = cw.tile([128, NFC], f32, name="ctmpb",
                                       tag="ctmpb")[:, 0:nf]
                        mty = [cw.tile([128, NFC], f32, name=f"mty{r}",
                                       tag=f"mty{r}")[:, 0:nf]
                               for r in range(3)]
                        tx2 = [cw.tile([128, NFC], f32, name=f"tx2{t}",
                                       tag=f"tx2{t}")[:, 0:nf]
                               for t in range(3)]
                        for i, r in enumerate((-1, 0, 1)):
                            nc.vector.tensor_scalar(
                                out=tmp, in0=dy_ap, scalar1=-1.0,
                                scalar2=float(1 + r), op0=OP.mult,
                                op1=OP.add)
                            nc.vector.tensor_scalar(
                                out=tmpb, in0=dy_ap, scalar1=float(1 - r),
                                scalar2=None, op0=OP.add)
                            nc.vector.tensor_tensor(
                                out=tmp, in0=tmp, in1=tmpb, op=OP.min)
                            nc.vector.scalar_tensor_tensor(
                                out=mty[i], in0=tmp, scalar=0.0,
                                in1=mk_sig, op0=OP.max, op1=OP.mult)
                        for i, t in enumerate((-1, 0, 1)):
                            nc.vector.tensor_scalar(
                                out=tmp, in0=dx_ap, scalar1=-1.0,
                                scalar2=float(1 + t), op0=OP.mult,
                                op1=OP.add)
                            nc.vector.tensor_scalar(
                                out=tmpb, in0=dx_ap, scalar1=float(1 - t),
                                scalar2=None, op0=OP.add)
                            nc.vector.tensor_tensor(
                                out=tx2[i], in0=tmp, in1=tmpb, op=OP.min)
                            nc.vector.tensor_scalar(
                                out=tx2[i], in0=tx2[i], scalar1=0.0,
                                scalar2=None, op0=OP.max)
                        for ri in range(3):
                            for ti in range(3):
                                rt = ri * 3 + ti
                                nc.gpsimd.tensor_tensor(
                                    out=coef3[:, rs, rt, :],
                                    in0=tx2[ti], in1=mty[ri], op=OP.mult)

                    def emit_row(rho, pout):
                        if True:
                            jr = rho % 4
                            samp = samp_slots[rho % 8]
                            for rt in range(KK):
                                r, t = rt // 3 - 1, rt % 3 - 1
                                for k in DVE_TAPS:
                                    ky, kx = k // 3, k % 3
                                    src = xt[2 + (kx - 1) + t]
                                    row = rho + PAD + (ky - 1) + r
                                    in0 = src[:, row * C:(row + 1) * C]
                                    col = rho * 81 + rt * 9 + k
                                    sl = samp[:, k * CO:(k + 1) * CO]
                                    if rt == 0:
                                        nc.vector.tensor_scalar(
                                            out=sl, in0=in0,
                                            scalar1=coef_sb[:, col:col + 1],
                                            scalar2=None, op0=OP.mult)
                                    else:
                                        nc.vector.scalar_tensor_tensor(
                                            out=sl, in0=in0,
                                            scalar=coef_sb[:, col:col + 1],
                                            in1=sl, op0=OP.mult,
                                            op1=OP.add)
                            k0 = ACT_TAPS[0]
                            sl3 = samp[:, k0 * CO:(k0 + NAT) * CO]
                            for rt in range(KK):
                                r, t = rt // 3 - 1, rt % 3 - 1
                                tm = tmps[rt % 3]
                                for ki, k in enumerate(ACT_TAPS):
                                    ky, kx = k // 3, k % 3
                                    src = xt[2 + (kx - 1) + t]
                                    row = rho + PAD + (ky - 1) + r
                                    in0 = src[:, row * C:(row + 1) * C]
                                    col = rho * 81 + rt * 9 + k
                                    dst = (samp[:, k * CO:(k + 1) * CO]
                                           if rt == 0 else
                                           tm[:, ki * CO:(ki + 1) * CO])
                                    nc.scalar.activation(
                                        dst, in0, AF.Copy,
                                        scale=coef_sb[:, col:col + 1])
                                if rt > 0:
                                    nc.gpsimd.tensor_tensor(
                                        out=sl3, in0=sl3,
                                        in1=tm[:], op=OP.add)
                            pt5 = stp.tile([128, 640], bf16, name="pt5",
                                           tag=f"pt5{rho % 2}")
                            for ci in range(5):
                                nc.tensor.transpose(
                                    pt5[:, ci * 128:(ci + 1) * 128],
                                    samp[:, ci * 128:(ci + 1) * 128],
                                    ident[:])
                            st = st_pool.tile([128, 640], bf16, name="st",
                                              tag=f"st{rho % 4}")
                            if rho % 2 == 0:
                                nc.scalar.activation(st[:], pt5[:], AF.Copy)
                            else:
                                nc.vector.tensor_copy(st[:], pt5[:])
                            po = pout[:, jr * CO:(jr + 1) * CO]
                            nc.tensor.matmul(po, ones_sb[:], mbias_sb[:],
                                             start=True, stop=False)
                            for ci in range(5):
                                p = 128 if ci < 4 else 64
                                nc.tensor.matmul(
                                    po, st[0:p, ci * 128:ci * 128 + 128],
                                    wm_sb[ci][:],
                                    start=False, stop=(ci == 4))

                    # fine-grained prep scheduling: each step is one PE
                    # om matmul (or xch-row / omT / coef emission); steps are
                    # drip-fed between combiner rows so PE's in-order queue
                    # never blocks sampT transposes for long.
                    coef_bounds = {0: (0, 4), 3: (4, 16), 7: (16, 32),
                                   11: (32, 48), 15: (48, 64)}
                    steps = []
                    coef_hi_after = []
                    xch_state = {"hi": 0}
                    om_tiles = {}
                    coef_hi = 0

                    def mk_xch(lo, hi):
                        return lambda: emit_xch_rows(lo, hi)

                    def mk_om(g4, k):
                        def f():
                            c16 = g4 // 4
                            if c16 not in om_tiles:
                                om_tiles[c16] = om_pool.tile(
                                    [27, 16 * W], bf16, name="omc",
                                    tag=f"omc{c16 % 2}")
                            emit_om_matmul(g4, om_tiles[c16], c16 * 16, k)
                        return f

                    def mk_omt(g4):
                        def f():
                            c16 = g4 // 4
                            emit_omt_granule(g4, om_tiles[c16], c16 * 16)
                        return f

                    def mk_coef(lo, hi):
                        return lambda: emit_coef_chunk(lo, hi)

                    for g4 in range(16):
                        need = min(XCH_ROWS, g4 * 4 + 6)
                        if need > xch_state["hi"]:
                            steps.append(mk_xch(xch_state["hi"], need))
                            coef_hi_after.append(coef_hi)
                            xch_state["hi"] = need
                        for k in range(KK):
                            steps.append(mk_om(g4, k))
                            coef_hi_after.append(coef_hi)
                        steps.append(mk_omt(g4))
                        coef_hi_after.append(coef_hi)
                        if g4 in coef_bounds:
                            lo, hi = coef_bounds[g4]
                            steps.append(mk_coef(lo, hi))
                            coef_hi = hi
                            coef_hi_after.append(coef_hi)

                    ptr = {"i": 0}

                    def run_steps(n):
                        i = ptr["i"]
                        stop = min(len(steps), i + n)
                        while i < stop:
                            steps[i]()
                            i += 1
                        ptr["i"] = i

                    def ensure_coef(rho):
                        i = ptr["i"]
                        while (i < len(steps)
                               and (i == 0 or coef_hi_after[i - 1] <= rho)):
                            steps[i]()
                            i += 1
                        ptr["i"] = i

                    for blk in range(16):
                        r0 = blk * 4
                        pout = op_.tile([128, 4 * CO], f32)
                        for jr in range(4):
                            rho = r0 + jr
                            ensure_coef(min(63, rho + 4))
                            emit_row(rho, pout)
                            run_steps(4)
                        osb = ou_pool.tile([128, 4 * CO], f32, name="osb",
                                           tag=f"osb{blk % 3}")
                        nc.vector.tensor_copy(osb[:], pout[:])
                        nc.sync.dma_start(
                            out=yo3[:, r0:r0 + 4, :],
                            in_=osb[:].rearrange("w (r c) -> w r c", r=4))

    return nc


def _split_multi_waits(nc, mybir, bass_rust):
    ctr = 0
    for fn in nc.m.functions:
        for bb in fn.blocks:
            new_insts = []
            for inst in bb.instructions:
                si = inst.sync_info
                if si is not None and len(si.on_wait) > 1:
                    waits = list(si.on_wait)
                    for w in waits[:-1]:
                        ctr += 1
                        nop = mybir.InstNoOp(name=f"I-waitsplit-{ctr}")
                        nop.engine = inst.engine
                        nop.sync_info = bass_rust.SyncInfo(
                            on_wait=[w], on_update=[])
                        new_insts.append(nop)
                    inst.sync_info = bass_rust.SyncInfo(
                        on_wait=[waits[-1]], on_update=list(si.on_update))
                new_insts.append(inst)
            bb.instructions = new_insts


def _get_nc(split=True, rep=1):
    key = ("nc", split, rep, ACT_TAPS)
    if key not in _CACHE:
        import concourse.mybir as mybir
        import bass_rust
        nc = _build(rep=rep)
        if split:
            _split_multi_waits(nc, mybir, bass_rust)
        _CACHE[key] = nc
    return _CACHE[key]


def make_in_maps(x, kernel, bias, offset_kernel, offset_bias):
    import ml_dtypes
    bf = ml_dtypes.bfloat16
    x = np.ascontiguousarray(np.asarray(x), np.float32)
    perm = list(range(0, 18, 2)) + list(range(1, 18, 2)) + list(range(18, 27))
    w0 = np.asarray(offset_kernel).reshape(KK, C, 27)[:, :, perm]
    woff_h = np.ascontiguousarray(
        w0.transpose(1, 0, 2).reshape(C, KK * 27), np.float32)
    wmain_h = np.ascontiguousarray(
        np.asarray(kernel).reshape(NKC, CO)).astype(bf)
    obias_h = np.ascontiguousarray(
        np.asarray(offset_bias)[perm].reshape(27, 1), np.float32)
    mbias_h = np.ascontiguousarray(
        np.asarray(bias).reshape(1, CO)).astype(bf)
    in_maps = []
    for core in range(8):
        b, half = core // 2, core % 2
        h0 = half * 64
        slab = np.zeros((ROWS, W, C), np.float32)
        lo, hi = h0 - PAD, h0 + 64 + PAD
        slo, shi = max(lo, 0), min(hi, H)
        slab[slo - lo: shi - lo] = x[b, slo:shi]
        # w-major with zero margins: [WS, ROWS, C]
        wslab = np.zeros((WS, ROWS, C), np.float32)
        wslab[WPAD:WPAD + W] = slab.transpose(1, 0, 2)
        in_maps.append({
            "xs": np.ascontiguousarray(
                wslab.reshape(WS, ROWS * C)).astype(bf),
            "woff": woff_h, "wmain": wmain_h,
            "obias": obias_h, "mbias": mbias_h,
        })
    return in_maps


def run(x, kernel, bias, offset_kernel, offset_bias, **kwargs):
    from concourse.bass_utils import run_bass_kernel_spmd
    nc = _get_nc()
    in_maps = make_in_maps(x, kernel, bias, offset_kernel, offset_bias)
    res = run_bass_kernel_spmd(nc, in_maps, core_ids=list(range(8)), **kwargs)
    out = np.empty((B, H, W, CO), np.float32)
    for core in range(8):
        b, half = core // 2, core % 2
        out[b, half * 64:half * 64 + 64] = (
            res.results[core]["yout"].reshape(64, W, CO))
    return out, res


def kernel(**inputs):
    out, _ = run(**inputs)
    return out

